# revision 45
# baseline (speedup 1.0000x reference)
"""Binarized VGG16 forward pass on 8 Trainium2 NeuronCores.

Strategy:
- Host (fp64): layer-0 conv (real-valued input, tiny: 0.45 GFLOP), its batch-norm
  stats and binarization. fp64 resolves the near-threshold sign decisions exactly.
- Device (8-way data parallel, 32 images/core): conv layers 1..12 + FC.
  All operands are exactly +-1 (bf16), so matmul accumulation in fp32 PSUM is
  exact integer arithmetic. BN is reformulated as a per-channel threshold:
  sign(bn(y)) == sign(y - t), t = mean - (beta/gamma)*sqrt(var+eps) (gamma>0).
  Batch stats are synced across cores with a tiny per-layer AllGather of
  (count, mean, M2) triples combined by bn_aggr. Maxpool commutes with sign.
- Conv biases and the FC bias cancel exactly through batchnorm and are dropped.
"""
import sys
sys.path.insert(0, '/opt/trn_rl_repo')
import numpy as np
import ml_dtypes

NCORES = 8
NIMG = 32          # images per core
EPS = 1e-5

# device conv layers: (cin, cout, H, W, pool_after, hbm_spill)
LAYERS = [
    (64,  64,  32, 32, True,  True),
    (64,  128, 16, 16, False, True),
    (128, 128, 16, 16, True,  True),
    (128, 256, 8,  8,  False, False),
    (256, 256, 8,  8,  False, False),
    (256, 256, 8,  8,  True,  False),
    (256, 512, 4,  4,  False, False),
    (512, 512, 4,  4,  False, False),
    (512, 512, 4,  4,  True,  False),
    (512, 512, 2,  2,  False, False),
    (512, 512, 2,  2,  False, False),
    (512, 512, 2,  2,  True,  False),
]
# (images per psum tile, block images, block rows)
GEOM = {32: (2, 1, 16), 16: (8, 2, 16), 8: (32, 8, 8), 4: (32, 32, 4), 2: (32, 32, 2)}

_TRACE = False
_DEBUG = False
_LAST_RESULTS = [None]
_LAST_INMAPS = [None]


def _build():
    import concourse.bacc as bacc
    import concourse.mybir as mybir
    import concourse.tile as tile

    dt = mybir.dt
    AF = mybir.ActivationFunctionType
    OP = mybir.AluOpType

    nc = bacc.Bacc(num_devices=NCORES)

    # ---- kernel I/O ----
    # b1p: partitions 0-63 hold the padded binarized layer-1 input; partitions
    # 64-127 hold the same data shifted left by 2 elements (tap-pairing trick:
    # one K=128 matmul computes both the dx=-1 and dx=+1 taps).
    b1p = nc.dram_tensor("b1p", [128, NIMG * 34 * 34], dt.bfloat16, kind="ExternalInput")
    wgt_ext, avec_ext = [], []
    for l, (cin, cout, H, W, pool, hbm) in enumerate(LAYERS):
        nw = 6 if l in (0, 1) else 9  # paired layers: 3 pair-taps + 3 singles
        wgt_ext.append(nc.dram_tensor(f"w{l}", [128 if l in (0, 1) else cin, nw * cout],
                                      dt.bfloat16, kind="ExternalInput"))
        nch = (cout + 127) // 128
        avec_ext.append(nc.dram_tensor(f"a{l}", [nch, min(cout, 128)], dt.float32, kind="ExternalInput"))
    wfc = nc.dram_tensor("wfc", [512, 10], dt.bfloat16, kind="ExternalInput")
    fcg = nc.dram_tensor("fcg", [10, 1], dt.float32, kind="ExternalInput")
    fcb = nc.dram_tensor("fcb", [10, 1], dt.float32, kind="ExternalInput")
    out_ext = nc.dram_tensor("out", [NIMG, 10], dt.float32, kind="ExternalOutput")
    dbg_ext = []
    if _DEBUG:
        for l, (cin, cout, H, W, pool, hbm) in enumerate(LAYERS):
            ncc = (cout + 127) // 128
            dbg_ext.append(nc.dram_tensor(f"dbg{l}", [min(cout, 128), 4 * ncc],
                                          dt.float32, kind="ExternalOutput"))
        dbg_fc = nc.dram_tensor("dbgfc", [10, NIMG + 4], dt.float32, kind="ExternalOutput")
        dbg_y = nc.dram_tensor("dbgy", [64, 2048], dt.float16, kind="ExternalOutput")
        dbg_trip = nc.dram_tensor("dbgtrip", [64, 3], dt.float32, kind="ExternalOutput")
        dbg_gt = nc.dram_tensor("dbggt", [64, NCORES * 3], dt.float32, kind="ExternalOutput")
        dbg_st = nc.dram_tensor("dbgst", [64, 64 * 6], dt.float32, kind="ExternalOutput")

    with tile.TileContext(nc) as tc:
        with (
            tc.tile_pool(name="pbq", bufs=2) as pbq,
            tc.tile_pool(name="pb", bufs=1) as pb,
            tc.tile_pool(name="pw", bufs=5) as pw,
            tc.tile_pool(name="pps", bufs=2, space="PSUM") as pps,
            tc.tile_pool(name="pspill", bufs=1) as pspill,
            tc.tile_pool(name="pstage", bufs=3) as pstage,
            tc.tile_pool(name="pstage2", bufs=3) as pstage2,
            tc.tile_pool(name="pbtmp", bufs=2) as pbtmp,
            tc.tile_pool(name="pmisc", bufs=1) as pmisc,
            tc.tile_pool(name="pdram", bufs=1, space="DRAM") as pdram,
        ):
            b_in = None      # list of tiles per cin chunk, padded layout
            in_geom = (34, 34)
            bq_tiles = {}    # layer-0 b1 quarters, loaded on demand

            # gathered-stats tile, reused by every layer: [p, chunk, rank, 3]
            # with the count slot (t=0) pre-set to 1.0 once
            gt_all = pmisc.tile([128, 4 * NCORES * 3], dt.float32, tag="gt", name="gt_all")
            nc.gpsimd.memset(gt_all[:].rearrange("p (k t) -> p k t", t=3)[:, :, 0], 1.0)

            for l, (cin, cout, H, W, pool, hbm) in enumerate(LAYERS):
                HP, WP = H + 2, W + 2
                M = min(cout, 128)
                ncin = (cin + 127) // 128
                ncout = (cout + 127) // 128
                KP = min(cin, 128)              # partitions per cin chunk
                NTOT = NIMG * H * W             # free size per cout chunk
                PS_IMGS, BLK_I, BLK_R = GEOM[H]
                PSF = PS_IMGS * H * W           # psum tile free size
                n_ps = NIMG // PS_IMGS
                NBLK = PSF // (BLK_I * BLK_R * W)
                ngrp = max(1, PSF // 512)       # bn_stats groups per psum tile
                grp_f = PSF // ngrp             # elements per group (<=512)
                n_grp_tot = ngrp * n_ps

                # ---- weights for this layer ----
                paired = l in (0, 1)
                # vtap: (row offset ky, col offset kx, K partitions, weight col idx)
                if paired:
                    vtaps = [(ky, 1, 64, 3 + ky) for ky in range(3)] + \
                            [(ky, 0, 128, ky) for ky in range(3)]
                else:
                    vtaps = [(ky, kx, KP, ky * 3 + kx) for ky in range(3) for kx in range(3)]
                nvt = len(vtaps)
                wts = []
                for k in range(ncin):
                    wkp = 128 if paired else KP
                    wt = pw.tile([wkp, nvt * cout], dt.bfloat16, tag="w", name=f"wt_{l}_{k}")
                    nc.sync.dma_start(wt[:], wgt_ext[l][k * 128:k * 128 + wkp, :])
                    wts.append(wt)

                # ---- output b tiles (padded; last layer: dense [128, NIMG]) ----
                last = (l == len(LAYERS) - 1)
                if pool:
                    HO, WO = H // 2, W // 2
                else:
                    HO, WO = H, W
                if last:
                    bo = [pb.tile([128, NIMG], dt.bfloat16, tag=f"bfc_{j}", name=f"bo_{l}_{j}")
                          for j in range(ncout)]
                else:
                    HPO, WPO = HO + 2, WO + 2
                    # layer 0's output feeds a paired layer: allocate 128
                    # partitions, upper half holds the shift-by-2 duplicate
                    MO = 128 if l == 0 else M
                    bo = []
                    for j in range(ncout):
                        t_ = pb.tile([MO, NIMG * HPO * WPO], dt.bfloat16,
                                     tag=f"b{(l + 1) % 2}_{j}", name=f"bo_{l}_{j}")
                        nc.gpsimd.memset(t_[:], 0.0)
                        bo.append(t_)

                # ---- spill destinations ----
                if hbm:
                    ydram = [pdram.tile([M, NTOT], dt.float16, name=f"ydram_{l}_{j}")
                             for j in range(ncout)]
                    spills = None
                else:
                    spills = [pspill.tile([M, NTOT], dt.float32, tag=f"spill{j}",
                                          name=f"sp_{l}_{j}") for j in range(ncout)]

                # ---- stats collection tiles ----
                sts = [pmisc.tile([M, n_grp_tot * 6], dt.float32, tag=f"st{j}",
                                  name=f"st_{l}_{j}") for j in range(ncout)]

                # ---- conv + per-tile stats + spill ----
                for j in range(ncout):
                    for ip in range(n_ps):
                        i0 = ip * PS_IMGS
                        if l == 0:
                            q = i0 // 8
                            if q not in bq_tiles:
                                t_ = pbq.tile([128, 8 * 34 * 34], dt.bfloat16, tag="bq", name=f"bq{q}")
                                nc.sync.dma_start(t_[:], b1p[:, q * 8 * 1156:(q + 1) * 8 * 1156])
                                bq_tiles[q] = t_
                        pt = pps.tile([M, PSF], dt.float32, tag="y", name=f"pt_{l}_{j}_{ip}")
                        pt4 = pt[:].rearrange("p (i h w) -> p i h w", i=PS_IMGS, h=H, w=W)
                        if hbm:
                            sg = pstage.tile([M, PSF], dt.float16, tag="stage", name=f"sg_{l}_{j}_{ip}")
                        mm = 0
                        for nb in range(NBLK):
                            bi = nb * BLK_I * BLK_R * W
                            ib0 = (bi // (H * W))
                            r0 = (bi // W) % H
                            for (ky, kx, kp, widx) in vtaps:
                                for k in range(ncin):
                                    if l == 0:
                                        # b1 quarters: 8 images each
                                        q = (i0 + ib0) // 8
                                        src, qoff = bq_tiles[q], (i0 + ib0) % 8
                                    else:
                                        src, qoff = b_in[k], i0 + ib0
                                    sv = src[0:kp, :].rearrange("p (i h w) -> p i h w",
                                                                h=in_geom[0], w=in_geom[1])
                                    rhs = sv[:, qoff:qoff + BLK_I,
                                             ky + r0:ky + r0 + BLK_R,
                                             kx:kx + W]
                                    outp = pt4[:, ib0:ib0 + BLK_I, r0:r0 + BLK_R, :]
                                    lhsT = wts[k][0:kp, widx * cout + j * 128:widx * cout + j * 128 + M]
                                    nc.tensor.matmul(outp, lhsT, rhs,
                                                     start=(mm % (nvt * ncin) == 0),
                                                     stop=(mm % (nvt * ncin) == nvt * ncin - 1))
                                    mm += 1
                            # per-bank stats as soon as this bank's group stops
                            if grp_f == BLK_I * BLK_R * W:
                                g = nb
                                nc.vector.bn_stats(sts[j][:, (ip * ngrp + g) * 6:(ip * ngrp + g + 1) * 6],
                                                   pt[:, g * grp_f:(g + 1) * grp_f])
                        if grp_f != BLK_I * BLK_R * W:
                            for g in range(ngrp):
                                nc.vector.bn_stats(sts[j][:, (ip * ngrp + g) * 6:(ip * ngrp + g + 1) * 6],
                                                   pt[:, g * grp_f:(g + 1) * grp_f])
                        if hbm:
                            nc.scalar.activation(sg[:], pt[:], AF.Copy)
                            nc.sync.dma_start(ydram[j][:, ip * PSF:(ip + 1) * PSF], sg[:])
                        else:
                            nc.scalar.activation(spills[j][:, ip * PSF:(ip + 1) * PSF], pt[:], AF.Copy)

                # ---- prefetch HBM-spilled y for the sign phase (overlaps the
                # stats collective; these DMAs depend only on the spill-outs) ----
                y2_tiles = {}
                if hbm:
                    for j in range(ncout):
                        for p in range(NTOT // PSF):
                            y2 = pstage2.tile([M, PSF], dt.float16, tag="stage2",
                                              name=f"y2_{l}_{j}_{p}")
                            nc.sync.dma_start(y2[:], ydram[j][:, p * PSF:(p + 1) * PSF])
                            y2_tiles[(j, p)] = y2

                # ---- cross-core stats ----
                # AllGather the per-core (mean, var) pairs directly; bn_aggr
                # combines them as count=1 triples (equal counts across cores),
                # the constant 1.0 count slots are pre-seeded in gt_all once.
                lagg = pmisc.tile([M, 2 * ncout], dt.float32, tag="lagg", name=f"lagg_{l}")
                for j in range(ncout):
                    nc.vector.bn_aggr(lagg[:, 2 * j:2 * j + 2],
                                      sts[j][:].rearrange("p (k t) -> p k t", t=3))
                cc_in = pdram.tile([M, 2 * ncout], dt.float32, name=f"ccin_{l}")
                cc_out = pdram.tile([NCORES, M, 2 * ncout], dt.float32, name=f"ccout_{l}",
                                    addr_space="Shared")
                nc.sync.dma_start(cc_in[:], lagg[:])
                nc.gpsimd.collective_compute(
                    "AllGather", OP.bypass,
                    replica_groups=[list(range(NCORES))],
                    ins=[cc_in[:].opt()], outs=[cc_out[:].opt()],
                )
                gagg = pmisc.tile([M, 2 * ncout], dt.float32, tag="gagg", name=f"gagg_{l}")
                gt4 = gt_all[0:M, :].rearrange("p (c r t) -> p c r t", r=NCORES, t=3)
                for j in range(ncout):
                    # issue on the ACT queue: this DMA waits for the collective,
                    # and on the sync queue it would stall later independent DMAs
                    nc.scalar.dma_start(
                        gt4[:, j, :, 1:3],
                        cc_out[:, :, 2 * j:2 * j + 2].rearrange("r c t -> c r t"))
                    nc.vector.bn_aggr(gagg[:, 2 * j:2 * j + 2], gt4[:, j, :, :])
                # t = m - A*sqrt(v+eps); negt = A*s - m
                ga3 = gagg[:].rearrange("p (c t) -> p c t", t=2)
                u = pmisc.tile([M, ncout], dt.float32, tag="u", name=f"u_{l}")
                nc.vector.tensor_scalar_add(u[:], ga3[:, :, 1], EPS)
                s0 = pmisc.tile([M, ncout], dt.float32, tag="s0", name=f"s0_{l}")
                nc.scalar.activation(s0[:], u[:], AF.Sqrt)
                for it in range(2):
                    rr = pmisc.tile([M, ncout], dt.float32, tag="rr", name=f"rr_{l}_{it}")
                    nc.vector.reciprocal(rr[:], s0[:])
                    ur = pmisc.tile([M, ncout], dt.float32, tag="ur", name=f"ur_{l}_{it}")
                    nc.vector.tensor_tensor(ur[:], u[:], rr[:], op=OP.mult)
                    s1 = pmisc.tile([M, ncout], dt.float32, tag="s1", name=f"s1_{l}_{it}")
                    nc.vector.tensor_tensor(s1[:], s0[:], ur[:], op=OP.add)
                    nc.vector.tensor_scalar_mul(s0[:], s1[:], 0.5)
                av = pmisc.tile([M, ncout], dt.float32, tag="av", name=f"av_{l}")
                nc.sync.dma_start(av[:], avec_ext[l][:].rearrange("c p -> p c"))
                As = pmisc.tile([M, ncout], dt.float32, tag="As", name=f"As_{l}")
                nc.vector.tensor_tensor(As[:], av[:], s0[:], op=OP.mult)
                negt = pmisc.tile([M, ncout], dt.float32, tag="negt", name=f"negt_{l}")
                nc.vector.tensor_tensor(negt[:], As[:], ga3[:, :, 0], op=OP.subtract)
                if _DEBUG:
                    if l == 0:
                        nc.sync.dma_start(dbg_trip[:], trip[:])
                        nc.sync.dma_start(dbg_gt[:], gt[:, 0:NCORES * 3])
                        nc.sync.dma_start(dbg_st[:], sts[0][:])
                    nc.sync.dma_start(dbg_ext[l][:, 0:2 * ncout], gagg[:])
                    nc.sync.dma_start(dbg_ext[l][:, 2 * ncout:3 * ncout], negt[:])
                    nc.sync.dma_start(dbg_ext[l][:, 3 * ncout:4 * ncout], lagg[:].rearrange("p (c t) -> p c t", t=2)[:, :, 0])

                # ---- sign (+ pool) ----
                if pool:
                    HO2, WO2 = H // 2, W // 2
                for j in range(ncout):
                    if hbm:
                        pieces = NTOT // PSF
                        pf = PSF
                    else:
                        pieces = 1
                        pf = NTOT
                    pimgs = pf // (H * W)
                    for p in range(pieces):
                        i0 = p * pimgs
                        if hbm:
                            y2 = y2_tiles[(j, p)]
                            src_ap = y2[:].rearrange("p (i h w) -> p i h w", h=H, w=W)
                        else:
                            src_ap = spills[j][:, p * pf:(p + 1) * pf].rearrange(
                                "p (i h w) -> p i h w", h=H, w=W)
                        if not pool:
                            bo5 = bo[j][0:M, :].rearrange("p (i h w) -> p i h w", h=H + 2, w=W + 2)
                            nc.scalar.activation(bo5[:, i0:i0 + pimgs, 1:1 + H, 1:1 + W],
                                                 src_ap, AF.Sign, bias=negt[:, j:j + 1])
                        else:
                            bt = pbtmp.tile([M, pf], dt.bfloat16, tag="btmp", name=f"bt_{l}_{j}_{p}")
                            nc.scalar.activation(bt[:].rearrange("p (i h w) -> p i h w", h=H, w=W),
                                                 src_ap, AF.Sign, bias=negt[:, j:j + 1])
                            mc = pbtmp.tile([M, pf // 2], dt.bfloat16, tag="mcol", name=f"mc_{l}_{j}_{p}")
                            b6 = bt[:].rearrange("p (i h w2 two) -> p i h w2 two", h=H, w2=W // 2, two=2)
                            mc4 = mc[:].rearrange("p (i h w) -> p i h w", h=H, w=WO2)
                            nc.vector.tensor_tensor(mc4, b6[:, :, :, :, 0], b6[:, :, :, :, 1], op=OP.max)
                            mc5 = mc[:].rearrange("p (i h2 two w) -> p i h2 two w", h2=HO2, two=2, w=WO2)
                            if last:
                                nc.vector.tensor_tensor(
                                    bo[j][:, i0:i0 + pimgs].rearrange("p (i h w) -> p i h w", h=1, w=1),
                                    mc5[:, :, :, 0, :], mc5[:, :, :, 1, :], op=OP.max)
                            else:
                                bo5 = bo[j][0:M, :].rearrange("p (i h w) -> p i h w", h=HO2 + 2, w=WO2 + 2)
                                nc.vector.tensor_tensor(bo5[:, i0:i0 + pimgs, 1:1 + HO2, 1:1 + WO2],
                                                        mc5[:, :, :, 0, :], mc5[:, :, :, 1, :], op=OP.max)

                if l == 0:
                    # build the shift-by-2 duplicate on partitions 64-127 for
                    # layer 1's tap pairing
                    FTOT = NIMG * (HO + 2) * (WO + 2)
                    nc.sync.dma_start(bo[0][64:128, 0:FTOT - 2], bo[0][0:64, 2:FTOT])
                b_in = bo
                in_geom = (HO + 2, WO + 2) if not last else None

            # ================= FC + bn1d + log_softmax =================
            wfcs = []
            for k in range(4):
                wt = pw.tile([128, 10], dt.bfloat16, tag="w", name=f"wfc_{k}")
                nc.sync.dma_start(wt[:], wfc[k * 128:(k + 1) * 128, :])
                wfcs.append(wt)
            ptfc = pps.tile([10, NIMG], dt.float32, tag="y", name="ptfc")
            for k in range(4):
                nc.tensor.matmul(ptfc[:], wfcs[k][:], b_in[k][:],
                                 start=(k == 0), stop=(k == 3))
            stfc = pmisc.tile([10, 6], dt.float32, tag="stfc", name="stfc")
            nc.vector.bn_stats(stfc[:], ptfc[:])
            lafc = pmisc.tile([10, 2], dt.float32, tag="lafc", name="lafc")
            nc.vector.bn_aggr(lafc[:], stfc[:].rearrange("p (k t) -> p k t", t=3))
            ccfi = pdram.tile([10, 2], dt.float32, name="ccfi")
            ccfo = pdram.tile([NCORES, 10, 2], dt.float32, name="ccfo", addr_space="Shared")
            nc.sync.dma_start(ccfi[:], lafc[:])
            nc.gpsimd.collective_compute("AllGather", OP.bypass,
                                         replica_groups=[list(range(NCORES))],
                                         ins=[ccfi[:].opt()], outs=[ccfo[:].opt()])
            gtf4 = gt_all[0:10, :].rearrange("p (c r t) -> p c r t", r=NCORES, t=3)
            nc.scalar.dma_start(gtf4[:, 0, :, 1:3], ccfo[:].rearrange("r c t -> c r t"))
            gafc = pmisc.tile([10, 2], dt.float32, tag="gafc", name="gafc")
            nc.vector.bn_aggr(gafc[:], gtf4[:, 0, :, :])
            ufc = pmisc.tile([10, 1], dt.float32, tag="ufc", name="ufc")
            nc.vector.tensor_scalar_add(ufc[:], gafc[:, 1:2], EPS)
            sfc = pmisc.tile([10, 1], dt.float32, tag="sfc", name="sfc")
            nc.scalar.activation(sfc[:], ufc[:], AF.Sqrt)
            for it in range(2):
                rrf = pmisc.tile([10, 1], dt.float32, tag="rrf", name=f"rrf{it}")
                nc.vector.reciprocal(rrf[:], sfc[:])
                urf = pmisc.tile([10, 1], dt.float32, tag="urf", name=f"urf{it}")
                nc.vector.tensor_tensor(urf[:], ufc[:], rrf[:], op=OP.mult)
                s1f = pmisc.tile([10, 1], dt.float32, tag="s1f", name=f"s1f{it}")
                nc.vector.tensor_tensor(s1f[:], sfc[:], urf[:], op=OP.add)
                nc.vector.tensor_scalar_mul(sfc[:], s1f[:], 0.5)
            rsf = pmisc.tile([10, 1], dt.float32, tag="rsf", name="rsf")
            nc.vector.reciprocal(rsf[:], sfc[:])
            gv = pmisc.tile([10, 1], dt.float32, tag="gv", name="gv")
            nc.sync.dma_start(gv[:], fcg[:])
            bv = pmisc.tile([10, 1], dt.float32, tag="bv", name="bv")
            nc.sync.dma_start(bv[:], fcb[:])
            gs = pmisc.tile([10, 1], dt.float32, tag="gs", name="gs")
            nc.vector.tensor_tensor(gs[:], gv[:], rsf[:], op=OP.mult)
            # z = (y - m) * gs + beta   (still [10, NIMG], class on partitions)
            zt = pmisc.tile([10, NIMG], dt.float32, tag="zt", name="zt")
            nc.vector.tensor_scalar(zt[:], ptfc[:], gafc[:, 0:1], gs[:],
                                    op0=OP.subtract, op1=OP.mult)
            nc.vector.tensor_scalar_add(zt[:], zt[:], bv[:])
            # transpose to [NIMG, 10] via DRAM bounce
            zb = pdram.tile([10, NIMG], dt.float32, name="zb")
            nc.sync.dma_start(zb[:], zt[:])
            zi = pmisc.tile([NIMG, 10], dt.float32, tag="zi", name="zi")
            nc.sync.dma_start(zi[:], zb[:].rearrange("c i -> i c"))
            mx = pmisc.tile([NIMG, 1], dt.float32, tag="mx", name="mx")
            nc.vector.tensor_reduce(mx[:], zi[:], axis=mybir.AxisListType.X, op=OP.max)
            nmx = pmisc.tile([NIMG, 1], dt.float32, tag="nmx", name="nmx")
            nc.vector.tensor_scalar_mul(nmx[:], mx[:], -1.0)
            ez = pmisc.tile([NIMG, 10], dt.float32, tag="ez", name="ez")
            se = pmisc.tile([NIMG, 1], dt.float32, tag="se", name="se")
            nc.scalar.activation(ez[:], zi[:], AF.Exp, bias=nmx[:], accum_out=se[:])
            lse = pmisc.tile([NIMG, 1], dt.float32, tag="lse", name="lse")
            nc.scalar.activation(lse[:], se[:], AF.Ln)
            fin = pmisc.tile([NIMG, 10], dt.float32, tag="fin", name="fin")
            nc.vector.tensor_scalar(fin[:], zi[:], mx[:], lse[:],
                                    op0=OP.subtract, op1=OP.subtract)
            nc.sync.dma_start(out_ext[:], fin[:])
            if _DEBUG:
                nc.scalar.activation(dbgt := pmisc.tile([10, NIMG], dt.float32, tag="dbgt", name="dbgt"), ptfc[:], AF.Copy)
                nc.sync.dma_start(dbg_fc[:, 0:NIMG], dbgt[:])
                nc.sync.dma_start(dbg_fc[:, NIMG:NIMG + 2], gafc[:])
                nc.sync.dma_start(dbg_fc[:, NIMG + 2:NIMG + 3], gs[:])
                nc.sync.dma_start(dbg_fc[:, NIMG + 3:NIMG + 4], sfc[:])

    return nc


def _host_layer0(x, w0, gamma0, beta0):
    """fp64 conv 3x3 (pad 1) + BN threshold + binarize for layer 0."""
    N, C, H, W = x.shape
    O = w0.shape[0]
    xp = np.zeros((N, C, H + 2, W + 2), np.float64)
    xp[:, :, 1:-1, 1:-1] = x.astype(np.float64)
    # im2col: [N*H*W, C*9] @ [C*9, O]
    win = np.lib.stride_tricks.sliding_window_view(xp, (3, 3), axis=(2, 3))
    # win: [N, C, H, W, 3, 3]
    A = win.transpose(0, 2, 3, 1, 4, 5).reshape(N * H * W, C * 9)
    Wm = np.sign(w0.astype(np.float64)).reshape(O, C * 9).T
    y = A @ Wm  # [N*H*W, O]
    m = y.mean(axis=0)
    v = y.var(axis=0)
    t = m - beta0.astype(np.float64) * np.sqrt(v + EPS) / gamma0.astype(np.float64)
    b = np.sign(y - t)  # [N*H*W, O]
    return b.reshape(N, H, W, O).transpose(0, 3, 1, 2)  # [N, O, H, W]


def kernel(x, conv_ws, conv_bs, bn_gammas, bn_betas, fc_w, fc_b, fc_gamma, fc_beta):
    from concourse.bass_utils import run_bass_kernel_spmd

    x = np.asarray(x)
    conv_ws = [np.asarray(w) for w in conv_ws]
    bn_gammas = [np.asarray(g) for g in bn_gammas]
    bn_betas = [np.asarray(b) for b in bn_betas]
    fc_w = np.asarray(fc_w)
    fc_gamma_np = np.asarray(fc_gamma)
    fc_beta_np = np.asarray(fc_beta)

    # ---- host layer 0 (fp64) ----
    b1 = _host_layer0(x, conv_ws[0], bn_gammas[0], bn_betas[0])  # [256, 64, 32, 32] of {-1,0,1}
    # pad and shard
    NB = x.shape[0]
    b1p_full = np.zeros((NB, 64, 34, 34), np.float32)
    b1p_full[:, :, 1:-1, 1:-1] = b1
    b1p_full = b1p_full.astype(ml_dtypes.bfloat16)

    # ---- weight/vec prep ----
    w_ins, a_ins = {}, {}
    for l, (cin, cout, H, W, pool, hbm) in enumerate(LAYERS):
        w = conv_ws[l + 1]
        if l in (0, 1):
            # paired layout: 3 pair-taps (kx=0 lower / kx=2 upper) + 3 singles (kx=1)
            wl = np.zeros((128, 6 * cout), np.float32)
            for ky in range(3):
                wl[0:64, ky * cout:(ky + 1) * cout] = np.sign(w[:, :, ky, 0]).T
                wl[64:128, ky * cout:(ky + 1) * cout] = np.sign(w[:, :, ky, 2]).T
                wl[0:64, (3 + ky) * cout:(4 + ky) * cout] = np.sign(w[:, :, ky, 1]).T
        else:
            wl = np.empty((cin, 9 * cout), np.float32)
            for tap in range(9):
                dy, dx = tap // 3, tap % 3
                wl[:, tap * cout:(tap + 1) * cout] = np.sign(w[:, :, dy, dx]).T
        w_ins[f"w{l}"] = wl.astype(ml_dtypes.bfloat16)
        A = (np.float64(bn_betas[l + 1]) / np.float64(bn_gammas[l + 1])).astype(np.float32)
        nch = (cout + 127) // 128
        a_ins[f"a{l}"] = A.reshape(nch, min(cout, 128))
    wfc_in = np.sign(fc_w.astype(np.float64)).astype(ml_dtypes.bfloat16).T.copy()  # [512, 10]

    in_maps = []
    for c in range(NCORES):
        lower = np.ascontiguousarray(
            b1p_full[c * NIMG:(c + 1) * NIMG].transpose(1, 0, 2, 3).reshape(64, -1))
        full = np.zeros((128, lower.shape[1]), lower.dtype)
        full[0:64] = lower
        full[64:128, 0:-2] = lower[:, 2:]
        im = {
            "b1p": full,
            "wfc": wfc_in,
            "fcg": fc_gamma_np.reshape(10, 1).astype(np.float32),
            "fcb": fc_beta_np.reshape(10, 1).astype(np.float32),
        }
        im.update(w_ins)
        im.update(a_ins)
        in_maps.append(im)

    nc = _build_module()
    res = run_bass_kernel_spmd(nc, in_maps, core_ids=list(range(NCORES)), trace=_TRACE)
    _LAST_RESULTS[0] = res
    _LAST_INMAPS[0] = in_maps
    out = np.concatenate([res.results[c]["out"] for c in range(NCORES)], axis=0)
    return out.astype(np.float32)


_NC_CACHE = [None]


def _build_module():
    if _NC_CACHE[0] is None:
        _NC_CACHE[0] = _build()
        _NC_CACHE[0].finalize()
    return _NC_CACHE[0]
